# revision 10
# baseline (speedup 1.0000x reference)
"""CurvatureAwareGCN Trainium2 kernel (8 NeuronCores, destination-sharded).

kernel(**inputs) takes FULL inputs from setup_inputs(), returns FULL [N,D] f32.

Design: core c owns destination rows [c*12500, (c+1)*12500) (no collectives).
Host does pure index/layout prep: edges CSR-sorted by (dest-block, src-chunk,
dest-row) and packed into 128-token wrapped tiles. Source features are
gathered on-device with dma_gather (int16 indices -> 4 chunk tables
[32768, 128ch bf16], channels [x(64) | curv | 1 | pad]; row 0 of each chunk
table is zeros, and padding tokens point there so they gather zeros).
Gate = sigmoid(|curv_dst - curv_src| * w + b) on DVE+ACT, msg = x * gate.
Segment-sum: one-hot (is_equal vs iota) matmul per 128-token tile accumulated
in PSUM per 128-row block; a ones-channel in the matmul RHS yields counts for
free. mean = sum/max(cnt,1), out = GELU(mean @ W_lin.T + b_lin) via PE
transpose + matmul, GELU+bias on ACT. Output is feature-major [64, 12544] per
core; the host transposes back.
"""

import os
import numpy as np
import ml_dtypes

import concourse.bass as bass
import concourse.tile as tile
from concourse import bacc, mybir
from concourse import bass_utils
from concourse import library_config
from concourse.masks import make_identity

N = 100000
E = 1600000
D = 64
P = 128
NCORES = 8
R_CORE = N // NCORES            # 12500 rows per core
NB = (R_CORE + P - 1) // P      # 98 blocks of 128 rows
R_PAD = NB * P                  # 12544
CH = 128                        # bf16 channels per table row (256B)
CS = 32767                      # chunk size (int16 index range, idx >= 1)
KCH = 4                         # chunk tables
GB = 8                          # blocks per stream-load group
MAXS = 16                       # max slot-columns per (block, chunk) segment

_bf16 = ml_dtypes.bfloat16

last_results = None


# ----------------------------------------------------------------- host plan
def _plan_segments(row, col):
    """Per-core sorted edge arrays + uniform padded segment sizes."""
    per_core = []
    counts = np.zeros((NCORES, NB, KCH), dtype=np.int64)
    for c in range(NCORES):
        lo, hi = c * R_CORE, (c + 1) * R_CORE
        m = (row >= lo) & (row < hi)
        r = (row[m] - lo).astype(np.int64)
        cc = col[m].astype(np.int64)
        blk = r // P
        chk = cc // CS
        order = np.lexsort((r, chk, blk))
        r, cc, blk, chk = r[order], cc[order], blk[order], chk[order]
        np.add.at(counts[c], (blk, chk), 1)
        per_core.append((r, cc, blk, chk))
    seg = counts.max(axis=0)
    seg_pad = ((seg + P - 1) // P) * P
    seg_pad[:, 0] = np.maximum(seg_pad[:, 0], P)   # >=1 tile per block
    assert int(seg_pad.max()) <= MAXS * P, f"segment too large: {seg_pad.max()}"
    return seg_pad, per_core


def _seg_offsets(seg_pad):
    seg_off = np.zeros((NB, KCH), dtype=np.int64)
    acc = 0
    for b in range(NB):
        for k in range(KCH):
            seg_off[b, k] = acc
            acc += int(seg_pad[b, k])
    return seg_off, acc


def _build_arenas(seg_pad, seg_off, tot, core_edges, curvature, core):
    r, cc, blk, chk = core_edges
    n_tiles = tot // P
    idx_arena = np.zeros((P, tot // 16), dtype=np.int16)
    rl_arena = np.full((P, n_tiles), -1.0, dtype=np.float32)
    crow_arena = np.zeros((P, n_tiles), dtype=np.float32)

    counts = np.zeros((NB, KCH), dtype=np.int64)
    np.add.at(counts, (blk, chk), 1)
    lo_c = core * R_CORE
    ptr = 0
    for b in range(NB):
        for k in range(KCH):
            n = int(counts[b, k])
            L = int(seg_pad[b, k])
            if L == 0:
                assert n == 0
                continue
            sl = slice(ptr, ptr + n)
            ptr += n
            idxs = np.zeros(L, dtype=np.int64)       # pad -> 0 (zero row)
            if n:
                idxs[:n] = cc[sl] - k * CS + 1
                assert idxs[:n].min() >= 1 and idxs[:n].max() <= CS
            off = int(seg_off[b, k])
            # wrapped [16, L/16], replicated to all 8 q7 cores
            w16 = idxs.reshape(L // 16, 16).T.astype(np.int16)
            idx_arena[:, off // 16 : (off + L) // 16] = np.tile(w16, (8, 1))
            if n:
                toks = np.arange(n)
                pcol = toks % P
                tcol = off // P + toks // P
                rl_arena[pcol, tcol] = (r[sl] % P).astype(np.float32)
                crow_arena[pcol, tcol] = curvature[lo_c + r[sl]]
    assert ptr == len(r)
    return idx_arena, rl_arena, crow_arena


# --------------------------------------------------------------- bass kernel
def _build_nc(seg_pad, seg_off, tot):
    n_tiles = tot // P
    nc = bacc.Bacc("TRN2", target_bir_lowering=False, debug=False,
                   num_devices=NCORES)

    tabs = [nc.dram_tensor(f"tab{k}", [CS + 1, CH], mybir.dt.bfloat16,
                           kind="ExternalInput") for k in range(KCH)]
    idxs_d = nc.dram_tensor("idxs", [P, tot // 16], mybir.dt.int16,
                            kind="ExternalInput")
    rl_d = nc.dram_tensor("rl", [P, n_tiles], mybir.dt.float32,
                          kind="ExternalInput")
    crow_d = nc.dram_tensor("crow", [P, n_tiles], mybir.dt.bfloat16,
                            kind="ExternalInput")
    wrep_d = nc.dram_tensor("wrep", [P, MAXS * D], mybir.dt.bfloat16,
                            kind="ExternalInput")
    brep_d = nc.dram_tensor("brep", [P, MAXS * D], mybir.dt.bfloat16,
                            kind="ExternalInput")
    wlinT_d = nc.dram_tensor("wlinT", [D, D], mybir.dt.float32,
                             kind="ExternalInput")
    blin_d = nc.dram_tensor("blin", [D, 1], mybir.dt.float32,
                            kind="ExternalInput")
    iota_d = nc.dram_tensor("iota", [P, P], mybir.dt.bfloat16,
                            kind="ExternalInput")
    outT_d = nc.dram_tensor("outT", [D, R_PAD], mybir.dt.float32,
                            kind="ExternalOutput")

    groups = [list(range(g, min(g + GB, NB))) for g in range(0, NB, GB)]

    with tile.TileContext(nc) as tc:
        with (
            tc.tile_pool(name="const", bufs=1) as cp,
            tc.tile_pool(name="gat", bufs=3) as gp,
            tc.tile_pool(name="work", bufs=3) as wp,
            tc.tile_pool(name="small", bufs=4) as sp,
            tc.tile_pool(name="pacc", bufs=2, space="PSUM") as pp,
            tc.tile_pool(name="pfin", bufs=2, space="PSUM") as pp2,
        ):
            nc.gpsimd.load_library(library_config.mlp)

            wrep_t = cp.tile([P, MAXS * D], mybir.dt.bfloat16)
            nc.sync.dma_start(out=wrep_t[:], in_=wrep_d[:])
            brep_t = cp.tile([P, MAXS * D], mybir.dt.bfloat16)
            nc.sync.dma_start(out=brep_t[:], in_=brep_d[:])
            wlinT_t = cp.tile([D, D], mybir.dt.float32)
            nc.sync.dma_start(out=wlinT_t[:], in_=wlinT_d[:])
            blin_t = cp.tile([D, 1], mybir.dt.float32)
            nc.sync.dma_start(out=blin_t[:], in_=blin_d[:])
            iota_t = cp.tile([P, P], mybir.dt.bfloat16)
            nc.sync.dma_start(out=iota_t[:], in_=iota_d[:])
            ident_t = cp.tile([P, P], mybir.dt.float32)
            make_identity(nc, ident_t[:])
            outT_t = cp.tile([D, R_PAD], mybir.dt.float32)

            for grp in groups:
                # group-wide rl / crow stream tiles
                g0 = int(seg_off[grp[0], 0]) // P
                g1 = (int(seg_off[grp[-1], KCH - 1])
                      + int(seg_pad[grp[-1], KCH - 1])) // P
                GT = g1 - g0
                rl_g = wp.tile([P, GT], mybir.dt.float32, tag="rl")
                nc.sync.dma_start(out=rl_g[:], in_=rl_d[:, g0:g1])
                crow_g = wp.tile([P, GT], mybir.dt.bfloat16, tag="crow")
                nc.sync.dma_start(out=crow_g[:], in_=crow_d[:, g0:g1])

                for b in grp:
                    psum_b = pp.tile([P, D + 1], mybir.dt.float32, tag="acc")
                    first = True
                    segs = [(k, int(seg_pad[b, k])) for k in range(KCH)
                            if int(seg_pad[b, k]) > 0]
                    for si, (k, L) in enumerate(segs):
                        S = L // P
                        off = int(seg_off[b, k])
                        scol = off // P - g0      # col offset in group streams
                        it = sp.tile([P, MAXS * P // 16], mybir.dt.int16,
                                     tag="idx")
                        nc.sync.dma_start(
                            out=it[:, : L // 16],
                            in_=idxs_d[:, off // 16 : (off + L) // 16],
                        )
                        G = gp.tile([P, MAXS, CH], mybir.dt.bfloat16, tag="G")
                        nc.gpsimd.dma_gather(
                            out_ap=G[:, :S, :],
                            in_ap=tabs[k][:],
                            idxs_ap=it[:, : L // 16],
                            num_idxs=L,
                            num_idxs_reg=L,
                            elem_size=CH,
                        )
                        # cdiff = |crow - ccol|
                        cdiff = sp.tile([P, MAXS], mybir.dt.bfloat16,
                                        tag="cdiff")
                        nc.vector.tensor_tensor(
                            out=cdiff[:, :S], in0=crow_g[:, scol : scol + S],
                            in1=G[:, :S, D], op=mybir.AluOpType.subtract,
                        )
                        nc.scalar.activation(cdiff[:, :S], cdiff[:, :S],
                                             mybir.ActivationFunctionType.Abs)
                        # A = cdiff*w + b ; gate = sigmoid(A)
                        cd = cdiff[:, :S]
                        cd_b = bass.AP(cd.tensor, cd.offset,
                                       list(cd.ap) + [[0, D]])
                        A1 = wp.tile([P, MAXS, D], mybir.dt.bfloat16, tag="A1")
                        nc.vector.tensor_tensor(
                            out=A1[:, :S, :], in0=cd_b,
                            in1=wrep_t[:, : S * D].rearrange(
                                "p (s d) -> p s d", d=D),
                            op=mybir.AluOpType.mult,
                        )
                        nc.vector.tensor_tensor(
                            out=A1[:, :S, :], in0=A1[:, :S, :],
                            in1=brep_t[:, : S * D].rearrange(
                                "p (s d) -> p s d", d=D),
                            op=mybir.AluOpType.add,
                        )
                        gate = wp.tile([P, MAXS, D], mybir.dt.bfloat16,
                                       tag="gate")
                        nc.scalar.activation(
                            gate[:, :S, :], A1[:, :S, :],
                            mybir.ActivationFunctionType.Sigmoid)
                        # msg = x * gate, plus ones channel for counts
                        msg = wp.tile([P, MAXS, D + 2], mybir.dt.bfloat16,
                                      tag="msg")
                        nc.vector.tensor_tensor(
                            out=msg[:, :S, 0:D], in0=G[:, :S, 0:D],
                            in1=gate[:, :S, :], op=mybir.AluOpType.mult,
                        )
                        nc.vector.tensor_copy(out=msg[:, :S, D],
                                              in_=G[:, :S, D + 1])
                        # one-hot scatter per 128-token tile
                        for ti in range(S):
                            oh = sp.tile([P, P], mybir.dt.bfloat16, tag="oh")
                            nc.vector.tensor_scalar(
                                out=oh[:], in0=iota_t[:],
                                scalar1=rl_g[:, scol + ti : scol + ti + 1],
                                scalar2=None,
                                op0=mybir.AluOpType.is_equal,
                            )
                            last = (si == len(segs) - 1) and (ti == S - 1)
                            nc.tensor.matmul(
                                out=psum_b[:, : D + 1],
                                lhsT=oh[:],
                                rhs=msg[:, ti, 0 : D + 1],
                                start=first, stop=last,
                            )
                            first = False
                    # mean + linear + gelu for block b
                    cntc = sp.tile([P, 1], mybir.dt.float32, tag="cnt")
                    nc.vector.tensor_scalar(
                        out=cntc[:], in0=psum_b[:, D : D + 1], scalar1=1.0,
                        scalar2=None, op0=mybir.AluOpType.max,
                    )
                    nc.vector.reciprocal(out=cntc[:], in_=cntc[:])
                    mean_sb = sp.tile([P, D], mybir.dt.float32, tag="mean")
                    nc.vector.tensor_scalar(
                        out=mean_sb[:], in0=psum_b[:, 0:D],
                        scalar1=cntc[:, 0:1], scalar2=None,
                        op0=mybir.AluOpType.mult,
                    )
                    tp = pp2.tile([D, P], mybir.dt.float32, tag="tp")
                    nc.tensor.transpose(out=tp[:], in_=mean_sb[:],
                                        identity=ident_t[:])
                    meanT = sp.tile([D, P], mybir.dt.float32, tag="meanT")
                    nc.vector.tensor_copy(out=meanT[:], in_=tp[:])
                    po = pp2.tile([D, P], mybir.dt.float32, tag="po")
                    nc.tensor.matmul(out=po[:], lhsT=wlinT_t[:], rhs=meanT[:],
                                     start=True, stop=True)
                    nc.scalar.activation(
                        outT_t[:, b * P : (b + 1) * P], po[:],
                        mybir.ActivationFunctionType.Gelu,
                        bias=blin_t[:, 0:1], scale=1.0,
                    )
            nc.sync.dma_start(out=outT_d[:], in_=outT_t[:])

    nc.compile()
    return nc


# ------------------------------------------------------------------- entry
def _make_tables(x, curvature):
    xb = x.astype(_bf16)
    cb = curvature.astype(_bf16)
    tabs = []
    for k in range(KCH):
        t = np.zeros((CS + 1, CH), dtype=_bf16)
        lo = k * CS
        n = min(CS, N - lo)
        if n > 0:
            t[1 : 1 + n, :D] = xb[lo : lo + n]
            t[1 : 1 + n, D] = cb[lo : lo + n]
            t[1 : 1 + n, D + 1] = 1.0
        tabs.append(t)
    return tabs


def kernel(x, curvature, W_curv, b_curv, W_lin, b_lin, edge_index):
    global last_results
    x = np.asarray(x, dtype=np.float32)
    curvature = np.asarray(curvature, dtype=np.float32)
    W_curv = np.asarray(W_curv, dtype=np.float32)
    b_curv = np.asarray(b_curv, dtype=np.float32)
    W_lin = np.asarray(W_lin, dtype=np.float32)
    b_lin = np.asarray(b_lin, dtype=np.float32)
    edge_index = np.asarray(edge_index)
    row = edge_index[0].astype(np.int64)
    col = edge_index[1].astype(np.int64)

    seg_pad, per_core = _plan_segments(row, col)
    seg_off, tot = _seg_offsets(seg_pad)

    tabs = _make_tables(x, curvature)
    w = W_curv[:, 0]
    wrep = np.tile(w, (P, MAXS)).astype(_bf16)
    brep = np.tile(b_curv, (P, MAXS)).astype(_bf16)
    wlinT = np.ascontiguousarray(W_lin.T).astype(np.float32)
    blin = b_lin.reshape(D, 1).astype(np.float32)
    iota = np.tile(np.arange(P, dtype=np.float32), (P, 1)).astype(_bf16)

    nc = _build_nc(seg_pad, seg_off, tot)

    in_maps = []
    for c in range(NCORES):
        idx_a, rl_a, crow_a = _build_arenas(
            seg_pad, seg_off, tot, per_core[c], curvature, c)
        m = {f"tab{k}": tabs[k] for k in range(KCH)}
        m.update({
            "idxs": idx_a,
            "rl": rl_a.astype(np.float32),
            "crow": crow_a.astype(_bf16),
            "wrep": wrep,
            "brep": brep,
            "wlinT": wlinT,
            "blin": blin,
            "iota": iota,
        })
        in_maps.append(m)

    trace = bool(int(os.environ.get("GCN_TRACE", "0")))
    res = bass_utils.run_bass_kernel_spmd(
        nc, in_maps, list(range(NCORES)), trace=trace)
    last_results = res
    global last_nc, last_in_maps
    last_nc = nc
    last_in_maps = in_maps

    out = np.zeros((N, D), dtype=np.float32)
    for c in range(NCORES):
        oT = np.asarray(res.results[c]["outT"], dtype=np.float32)
        out[c * R_CORE : (c + 1) * R_CORE] = oT.T[:R_CORE]
    return out
